# revision 6
# baseline (speedup 1.0000x reference)
"""GAT-style dense-mask attention (gnn_message_passing) on 8 trn2 cores.

Sharding v2: core c owns heads {2*(c//4), 2*(c//4)+1} and query rows
[1024*(c%4), +1024).  vs v1 (1 head x 2048 rows) this halves the
adjacency DMA (8.4MB vs 16.8MB/core) and, with bf16 inputs, halves the
input DMA; every core computes support for ALL nodes for its 2 heads
(no collectives - the extra PE work is cheaper than an all-gather's
~10us/step latency floor).

Math per core (node order rolled so own rows come first):
  support_h = X @ Wh_h           [4096, 128]  (bf16 operands, f32 psum)
  f1_h = X @ (Wh u), f2_h = X @ (Wh v)
  q_h[j,i] = max(e^{f1[j]} * e^{0.8 f2[i]}, e^{0.2 f1[j]})   (softmax
     shift-invariance: whole column i divided by e^{0.2 f2[i]})
  p_h = q_h * adj[i,j]   (exp underflow replaces the -1e30 trick)
  out_h[i,:] = (p_h.T @ [supp_h|1])[:, :128] / denom + X[i] @ proj_w_h
               + bias_h + proj_b_h

Elementwise paths per (j-chunk-pair, head) unit [128, 2*1024]:
  d-path (DVE): p = (Gb *. e1[j]) max. e1p[j]    one dual-op
     tensor_scalar per half at 4x, then mask TT at 2x.
  r-path (ACT): max in LOG space via Relu:
     u = Relu(0.8*F2b + 0.8 f1[j]);  p = Exp(u + 0.2 f1[j])
     then mask TT only (1.224us) on DVE.  Balances ACT vs DVE.
  Optional gpsimd offload knobs for mask TT / combine ts.

The v1 fused MAX,MULTIPLY scalar_tensor_tensor ran at 1x (2286ns/tile);
the split ts(4x)+tt(2x) form is ~1.6x faster for the same work.

Aggregation: p-chunks as PE weights (FWL, bf16), rhs = [supp_h|1] 129
cols, persistent psum accumulation across all 32 j-chunks; per-head acc
banks are not shared across heads (matmul start=True zeroes the whole
bank).
"""

import os

import ml_dtypes
import numpy as np

N = 4096
IN = 512
D = 128
H = 4
NCORES = 8
RPC = 1024            # query rows per core
HPC = 2               # heads per core
JCH = N // 128        # 32 source-node chunks
ICH = RPC // 128      # 8 query-row chunks
JG = 8                # j-chunks per group
NG = JCH // JG        # 4 groups
NPAIR = JG // 2       # 4 j-chunk pairs per group
SUPW = 2 * (D + 1)    # per-chunk supp row: [h0|1|h1|1] = 258

_cache = {}


def _build_program():
    import concourse.bacc as bacc
    import concourse.mybir as mybir
    import concourse.tile as tile

    f32 = mybir.dt.float32
    bf16 = mybir.dt.bfloat16
    Exp = mybir.ActivationFunctionType.Exp
    Relu = mybir.ActivationFunctionType.Relu
    add = mybir.AluOpType.add
    mult = mybir.AluOpType.mult
    amax = mybir.AluOpType.max

    n_r = int(os.environ.get("KV2_R", "10"))          # r-path unit count
    n_gps_tt = int(os.environ.get("KV2_GPS_TT", "0"))  # masks on gpsimd
    n_gps_ts = int(os.environ.get("KV2_GPS_TS", "0"))  # combines on gpsimd
    f12_act = os.environ.get("KV2_F12_ACT", "0") == "1"
    supp_dve = os.environ.get("KV2_SUPP_DVE", "0") == "1"
    NU = NG * NPAIR * HPC  # 32 elementwise units
    r_set = {int((i + 0.5) * NU / n_r) for i in range(n_r)} if n_r else set()
    d_units = [u for u in range(NU) if u not in r_set]
    gps_tt_set = {d_units[int((i + 0.5) * len(d_units) / n_gps_tt)]
                  for i in range(n_gps_tt)} if n_gps_tt else set()
    gps_ts_set = {d_units[int((i + 0.5) * len(d_units) / n_gps_ts)]
                  for i in range(n_gps_ts)} if n_gps_ts else set()

    nc = bacc.Bacc(
        "TRN2",
        target_bir_lowering=False,
        debug=False,
        enable_asserts=False,
        num_devices=NCORES,
    )

    adjT = nc.dram_tensor("adjT", [N, RPC], bf16, kind="ExternalInput").ap()
    inpT = nc.dram_tensor("inpT", [IN, N], bf16, kind="ExternalInput").ap()
    wh2 = nc.dram_tensor("wh2", [IN, 2 * D], bf16, kind="ExternalInput").ap()
    whT2 = nc.dram_tensor("whT2", [D, 2 * IN], bf16, kind="ExternalInput").ap()
    uv4 = nc.dram_tensor("uv4", [D, 4], bf16, kind="ExternalInput").ap()
    pwh = nc.dram_tensor("pwh", [IN, 2 * D], bf16, kind="ExternalInput").ap()
    br = nc.dram_tensor("br", [2, 2 * D], f32, kind="ExternalInput").ap()
    outb = nc.dram_tensor("outb", [RPC, 2 * D], f32, kind="ExternalOutput").ap()

    with tile.TileContext(nc) as tc:
        with tc.tile_pool(name="persist", bufs=1) as persist, \
             tc.tile_pool(name="adjp", bufs=3) as adjp, \
             tc.tile_pool(name="pbufp", bufs=6) as pbufp, \
             tc.tile_pool(name="up", bufs=4) as up, \
             tc.tile_pool(name="epp", bufs=2) as epp:
            # ---- persistent tiles ----
            supp_g = [persist.tile([128, JG * SUPW], bf16, tag=f"supp{g}",
                                   name=f"supp{g}") for g in range(NG)]
            f12_g = [persist.tile([128, JG * 4], f32, tag=f"f12{g}",
                                  name=f"f12{g}") for g in range(NG)]
            # per-(g) column factors, cols h*JG+jo
            e1_g = [persist.tile([128, HPC * JG], f32, tag=f"e1{g}",
                                 name=f"e1{g}") for g in range(NG)]
            e1p_g = [persist.tile([128, HPC * JG], f32, tag=f"e1p{g}",
                                  name=f"e1p{g}") for g in range(NG)]
            f08_g = [persist.tile([128, HPC * JG], f32, tag=f"f08{g}",
                                  name=f"f08{g}") for g in range(NG)]
            f02_g = [persist.tile([128, HPC * JG], f32, tag=f"f02{g}",
                                  name=f"f02{g}") for g in range(NG)]
            F2b = persist.tile([128, HPC * RPC], f32)   # f2 bcast per head
            Gb = persist.tile([128, HPC * RPC], bf16)   # e^{0.8 f2} per head
            res = persist.tile([128, ICH * 2 * D], f32)  # residual+bias
            # inputsT resident: 16 tiles [128, 1024] (kc, blk)
            it = {}
            for kc in range(4):
                for blk in range(4):
                    it[(kc, blk)] = persist.tile(
                        [128, 1024], bf16, tag=f"it{kc}_{blk}",
                        name=f"it{kc}_{blk}")
            rhs_kc = [persist.tile([128, 260], bf16, tag=f"rhs{kc}",
                                   name=f"rhs{kc}") for kc in range(4)]
            pwh_sb = [persist.tile([128, 2 * D], bf16, tag=f"pwh{kc}",
                                   name=f"pwh{kc}") for kc in range(4)]
            whT2_sb = persist.tile([D, 2 * IN], bf16)
            uv4_sb = persist.tile([D, 4], bf16)
            br_sb = persist.tile([1, 4 * D], f32)
            br_row = persist.tile([1, 2 * D], bf16)
            bsum = persist.tile([1, 2 * D], f32)
            ones1 = persist.tile([1, 128], bf16)
            f2row = persist.tile([1, HPC * RPC], f32)

            for g in range(NG):
                nc.gpsimd.memset(supp_g[g], 1.0)
            nc.vector.memset(ones1, 1.0)

            # ---- input DMAs ----
            nc.gpsimd.dma_start(out=whT2_sb, in_=whT2)
            nc.gpsimd.dma_start(out=uv4_sb, in_=uv4)
            nc.scalar.dma_start(out=br_sb[0:1, 0:2 * D], in_=br[0:1, :])
            nc.scalar.dma_start(out=br_sb[0:1, 2 * D:4 * D], in_=br[1:2, :])
            for kc in range(4):
                nc.scalar.dma_start(
                    out=rhs_kc[kc][:, 4:260],
                    in_=wh2[kc * 128:(kc + 1) * 128, :])
                nc.scalar.dma_start(
                    out=pwh_sb[kc], in_=pwh[kc * 128:(kc + 1) * 128, :])
            # own-row input blocks first (gate for f2/residual), then rest
            for kc in range(4):
                nc.sync.dma_start(
                    out=it[(kc, 0)], in_=inpT[kc * 128:(kc + 1) * 128, 0:1024])
            for blk in range(1, 4):
                for kc in range(4):
                    nc.sync.dma_start(
                        out=it[(kc, blk)],
                        in_=inpT[kc * 128:(kc + 1) * 128,
                                 blk * 1024:(blk + 1) * 1024])

            # (bias + proj_b) as a k=1 matmul row
            nc.vector.tensor_add(bsum, br_sb[0:1, 0:2 * D],
                                 br_sb[0:1, 2 * D:4 * D])
            nc.vector.tensor_copy(out=br_row, in_=bsum)

            # ---- early phase: w12, f2 rows, residual ----
            with tc.tile_pool(name="eps", bufs=2, space="PSUM") as eps:
                # w12: w1_h|w2_h column pieces per k-chunk -> rhs cols 0:4
                for h in range(HPC):
                    for kc in range(4):
                        wps = eps.tile([128, 2], f32, tag="w12ps")
                        nc.tensor.matmul(
                            wps,
                            whT2_sb[:, h * IN + kc * 128:h * IN + (kc + 1) * 128],
                            uv4_sb[:, 2 * h:2 * h + 2],
                            start=True, stop=True)
                        nc.vector.tensor_copy(
                            out=rhs_kc[kc][:, 2 * h:2 * h + 2], in_=wps)
                # f2 rows for own 1024 rows (inpT cols 0:1024)
                for h in range(HPC):
                    for nh in range(2):
                        f2ps = eps.tile([1, 512], f32, tag="f2ps")
                        for kc in range(4):
                            nc.tensor.matmul(
                                f2ps,
                                rhs_kc[kc][:, 2 * h + 1:2 * h + 2],
                                it[(kc, 0)][:, nh * 512:(nh + 1) * 512],
                                start=(kc == 0), stop=(kc == 3))
                        nc.scalar.copy(
                            out=f2row[0:1, h * RPC + nh * 512:
                                      h * RPC + (nh + 1) * 512],
                            in_=f2ps)
                # residual: own rows x [proj_w | +bias row]
                for oc in range(ICH):
                    rps = eps.tile([128, 2 * D], f32, tag="rps")
                    for kc in range(4):
                        nc.tensor.matmul(
                            rps, it[(kc, 0)][:, oc * 128:(oc + 1) * 128],
                            pwh_sb[kc], start=(kc == 0), stop=False)
                    nc.tensor.matmul(rps, ones1, br_row,
                                     start=False, stop=True)
                    nc.scalar.copy(
                        out=res[:, oc * 2 * D:(oc + 1) * 2 * D], in_=rps)

            for h in range(HPC):
                nc.gpsimd.partition_broadcast(
                    F2b[:, h * RPC:(h + 1) * RPC],
                    f2row[0:1, h * RPC:(h + 1) * RPC])
                nc.scalar.activation(
                    Gb[:, h * RPC:(h + 1) * RPC],
                    F2b[:, h * RPC:(h + 1) * RPC], Exp, scale=0.8)

            # ---- stage 1: support + f1/f2 for all 32 node chunks ----
            with tc.tile_pool(name="s1p", bufs=3, space="PSUM") as s1p:
                for g in range(NG):
                    for jo in range(JG):
                        nchunk = g * JG + jo
                        blk, col = nchunk // 8, nchunk % 8
                        ps = s1p.tile([128, 260], f32, tag="s1ps")
                        for kc in range(4):
                            nc.tensor.matmul(
                                ps,
                                it[(kc, blk)][:, col * 128:(col + 1) * 128],
                                rhs_kc[kc],
                                start=(kc == 0), stop=(kc == 3))
                        # supp copy: psum cols 4:260 -> [h0(128)|.|h1(128)|.]
                        so = supp_g[g][:, jo * SUPW:(jo + 1) * SUPW].rearrange(
                            "p (h q) -> p h q", q=D + 1)[:, :, 0:D]
                        si = ps[:, 4:260].rearrange("p (h q) -> p h q", q=D)
                        if supp_dve:
                            nc.vector.tensor_copy(out=so, in_=si)
                        else:
                            nc.scalar.copy(out=so, in_=si)
                        if f12_act:
                            nc.scalar.copy(
                                out=f12_g[g][:, jo * 4:(jo + 1) * 4],
                                in_=ps[:, 0:4])
                        else:
                            nc.vector.tensor_copy(
                                out=f12_g[g][:, jo * 4:(jo + 1) * 4],
                                in_=ps[:, 0:4])
                    # column factors for this group
                    f12v = f12_g[g].rearrange("p (j c) -> p j c", c=4)
                    for h in range(HPC):
                        dst = slice(h * JG, (h + 1) * JG)
                        o1 = e1_g[g][:, dst].rearrange(
                            "p (j o) -> p j o", o=1)
                        o2 = e1p_g[g][:, dst].rearrange(
                            "p (j o) -> p j o", o=1)
                        o3 = f08_g[g][:, dst].rearrange(
                            "p (j o) -> p j o", o=1)
                        o4 = f02_g[g][:, dst].rearrange(
                            "p (j o) -> p j o", o=1)
                        f1v = f12v[:, :, 2 * h:2 * h + 1]
                        nc.scalar.activation(o1, f1v, Exp)
                        nc.scalar.activation(o2, f1v, Exp, scale=0.2)
                        nc.vector.tensor_scalar(
                            out=o3, in0=f1v, scalar1=0.8, scalar2=None,
                            op0=mult)
                        nc.vector.tensor_scalar(
                            out=o4, in0=f1v, scalar1=0.2, scalar2=None,
                            op0=mult)

            # ---- stage 2: attention units + aggregation ----
            # acc banks: head h, i-chunk ic -> slot s = h*ICH+ic;
            # 3 banks per head (slots not shared across heads: start=True
            # zeroes the whole bank)
            acc_tiles = []
            acc_frees = []
            for bk in range(6):
                a_t, a_free = tc.tile([128, 512], f32, space="PSUM",
                                      name=f"acc{bk}")
                acc_tiles.append(a_t)
                acc_frees.append(a_free)

            def acc_slot(h, ic):
                s = h * ICH + ic
                lb = s % ICH  # local slot within head: 0..7
                return acc_tiles[h * 3 + lb // 3], (lb % 3) * (D + 1)

            uidx = 0
            for g in range(NG):
                for pr in range(NPAIR):
                    jc0 = g * JG + 2 * pr
                    adj_t = adjp.tile([128, 2 * RPC], bf16, tag="adj")
                    for half in range(2):
                        jc = jc0 + half
                        nc.sync.dma_start(
                            out=adj_t[:, half * RPC:(half + 1) * RPC],
                            in_=adjT[jc * 128:(jc + 1) * 128, :])
                    for h in range(HPC):
                        u = uidx
                        uidx += 1
                        p_t = pbufp.tile([128, 2 * RPC], bf16, tag="pbuf")
                        hs = slice(h * RPC, (h + 1) * RPC)
                        for half in range(2):
                            jo = 2 * pr + half
                            sl = slice(half * RPC, (half + 1) * RPC)
                            col = h * JG + jo
                            if u in r_set:
                                # ACT path: max in log space
                                u_t = up.tile([128, RPC], f32, tag="u",
                                              name="u_t")
                                nc.scalar.activation(
                                    u_t, F2b[:, hs], Relu,
                                    bias=f08_g[g][:, col:col + 1], scale=0.8)
                                nc.scalar.activation(
                                    p_t[:, sl], u_t, Exp,
                                    bias=f02_g[g][:, col:col + 1])
                            elif u in gps_ts_set:
                                nc.gpsimd.tensor_scalar(
                                    out=p_t[:, sl], in0=Gb[:, hs],
                                    scalar1=e1_g[g][:, col:col + 1],
                                    scalar2=e1p_g[g][:, col:col + 1],
                                    op0=mult, op1=amax)
                            else:
                                nc.vector.tensor_scalar(
                                    out=p_t[:, sl], in0=Gb[:, hs],
                                    scalar1=e1_g[g][:, col:col + 1],
                                    scalar2=e1p_g[g][:, col:col + 1],
                                    op0=mult, op1=amax)
                        if u in gps_tt_set:
                            nc.gpsimd.tensor_mul(p_t, adj_t, p_t)
                        else:
                            nc.vector.tensor_mul(p_t, adj_t, p_t)
                        # aggregation: p-chunks as weights, rhs [supp|1]
                        first = (g == 0 and pr == 0)
                        last = (g == NG - 1 and pr == NPAIR - 1)
                        for half in range(2):
                            jo = 2 * pr + half
                            rhs = supp_g[g][:, jo * SUPW + h * (D + 1):
                                            jo * SUPW + (h + 1) * (D + 1)]
                            for ic in range(ICH):
                                acc, off = acc_slot(h, ic)
                                lhsT = p_t[:, half * RPC + ic * 128:
                                           half * RPC + (ic + 1) * 128]
                                nc.tensor.matmul(
                                    acc[:, off:off + D + 1], lhsT, rhs,
                                    start=(first and half == 0
                                           and ic % 3 == 0),
                                    stop=(last and half == 1
                                          and (ic % 3 == 2 or ic == ICH - 1)),
                                )

            # ---- epilogue: normalize + residual + store ----
            for h in range(HPC):
                for b3 in range(3):
                    ics = [ic for ic in range(b3 * 3, min(b3 * 3 + 3, ICH))]
                    nsl = len(ics)
                    acc = acc_tiles[h * 3 + b3]
                    accv = acc[:, 0:nsl * (D + 1)].rearrange(
                        "p (s q) -> p s q", q=D + 1)
                    den = epp.tile([128, 4], f32, tag="den", name="den")
                    denv = den[:, 0:nsl].rearrange("p (s o) -> p s o", o=1)
                    nc.vector.tensor_scalar(
                        out=denv, in0=accv[:, :, D:D + 1],
                        scalar1=1e-30, scalar2=None, op0=amax)
                    rc = epp.tile([128, 4], f32, tag="rc")
                    nc.vector.reciprocal(rc[:, 0:nsl], den[:, 0:nsl])
                    for s3, ic in enumerate(ics):
                        of = epp.tile([128, D], f32, tag="of")
                        nc.vector.scalar_tensor_tensor(
                            of, in0=acc[:, s3 * (D + 1):s3 * (D + 1) + D],
                            scalar=rc[:, s3:s3 + 1],
                            in1=res[:, ic * 2 * D + h * D:
                                    ic * 2 * D + (h + 1) * D],
                            op0=mult, op1=add)
                        nc.sync.dma_start(
                            out=outb[ic * 128:(ic + 1) * 128,
                                     h * D:(h + 1) * D],
                            in_=of)
            for a_free in reversed(acc_frees):
                a_free()

    nc.compile()
    return nc


def _get_program():
    key = ("prog2",
           os.environ.get("KV2_R", "10"),
           os.environ.get("KV2_GPS_TT", "0"),
           os.environ.get("KV2_GPS_TS", "0"),
           os.environ.get("KV2_F12_ACT", "0"),
           os.environ.get("KV2_SUPP_DVE", "0"))
    if key not in _cache:
        _cache[key] = _build_program()
    return _cache[key]


def kernel(inputs, adjacency, weight, weight_u, weight_v, bias, proj_w, proj_b):
    from concourse.bass_utils import run_bass_kernel_spmd

    bf = ml_dtypes.bfloat16
    inputs = np.ascontiguousarray(np.asarray(inputs, np.float32))
    adjacency = np.asarray(adjacency, np.float32)
    weight = np.asarray(weight, np.float32)
    weight_u = np.asarray(weight_u, np.float32)
    weight_v = np.asarray(weight_v, np.float32)
    bias = np.asarray(bias, np.float32).reshape(1, H * D)
    proj_w = np.asarray(proj_w, np.float32)
    proj_b = np.asarray(proj_b, np.float32).reshape(H * D)

    nc = _get_program()

    in_maps = []
    for c in range(NCORES):
        h2 = c // 4           # head pair: heads 2*h2, 2*h2+1
        r0 = (c % 4) * RPC
        hs = slice(2 * h2 * D, (2 * h2 + 2) * D)
        rolled = np.roll(inputs, -r0, axis=0)
        inpT_c = np.ascontiguousarray(rolled.T).astype(bf)
        adjT_c = np.ascontiguousarray(
            np.roll(adjacency[r0:r0 + RPC, :], -r0, axis=1).T
        ).astype(bf)  # exact: adjacency is 0.0/1.0
        wh2_c = np.ascontiguousarray(weight[:, hs]).astype(bf)
        whT2_c = np.ascontiguousarray(
            np.concatenate([weight[:, 2 * h2 * D:(2 * h2 + 1) * D].T,
                            weight[:, (2 * h2 + 1) * D:(2 * h2 + 2) * D].T],
                           axis=1)).astype(bf)
        uv4_c = np.ascontiguousarray(
            np.concatenate([weight_u[2 * h2], weight_v[2 * h2],
                            weight_u[2 * h2 + 1], weight_v[2 * h2 + 1]],
                           axis=1)).astype(bf)
        in_maps.append({
            "adjT": adjT_c,
            "inpT": inpT_c,
            "wh2": wh2_c,
            "whT2": whT2_c,
            "uv4": uv4_c,
            "pwh": np.ascontiguousarray(proj_w[:, hs]).astype(bf),
            "br": np.ascontiguousarray(
                np.stack([bias[0, hs], proj_b[hs]], axis=0)),
        })

    trace = os.environ.get("KERNEL_TRACE", "0") == "1"
    results = run_bass_kernel_spmd(
        nc, in_maps, core_ids=list(range(NCORES)), trace=trace)
    _cache["last_results"] = results

    out = np.empty((N, H * D), np.float32)
    for c in range(NCORES):
        h2 = c // 4
        r0 = (c % 4) * RPC
        out[r0:r0 + RPC, 2 * h2 * D:(2 * h2 + 2) * D] = results.results[c]["outb"]
    return out


# revision 15
# speedup vs baseline: 1.0212x; 1.0212x over previous
"""GAT-style dense-mask attention (gnn_message_passing) on 8 trn2 cores.

Sharding v2: core c owns heads {2*(c//4), 2*(c//4)+1} and query rows
[1024*(c%4), +1024).  vs v1 (1 head x 2048 rows) this halves the
adjacency DMA (8.4MB vs 16.8MB/core) and with bf16 inputs halves the
input DMA; every core computes support for ALL nodes for its 2 heads
(no collectives - an all-gather has a ~10us/step latency floor).

Math per core (node order rolled so own rows come first):
  support_h = X @ Wh_h           [4096, 128]  (bf16 operands, f32 psum)
  f1_h = X @ (Wh u), f2_h = X @ (Wh v)
  q_h[j,i] = max(e^{f1[j]} * e^{0.8 f2[i]}, e^{0.2 f1[j]})   (column i
     of the softmax divided by e^{0.2 f2[i]} - cancels in softmax)
  p_h = q_h * adj[i,j]   (exp underflow replaces the -1e30 trick)
  out_h[i,:] = (p_h.T @ [supp_h|1])[:, :128] / denom + X[i] @ proj_w_h
               + bias_h + proj_b_h

Elementwise unit = (j-chunk-pair, head): [128, 2*1024].  Three paths:
  E-path: ACT exp (t = e^{0.8 F2b + f1[j]}) then ONE fused DVE STT
     (t max e1p[j]) * adj per half - all-bf16 operands.
  d-path (pure DVE): dual-op tensor_scalar (Gb *. e1) max. e1p, then
     mask TT.
  g-path: same as d-path but on gpsimd (both ts and mask) to offload.
Emission is two-pass: ALL stage-1 PE matmuls first (PE never waits on
stage 2), then per-group copies + elementwise + aggregation matmuls so
DVE/ACT start group 0 at ~t=8us instead of after the whole stage 1.

Aggregation: p-chunks as PE weights (FWL bf16), rhs = [supp_h|1] 129
cols, persistent psum accumulation across all 32 j-chunks; acc banks
are per-head (matmul start=True zeroes the whole bank).
"""

import os

import ml_dtypes
import numpy as np

N = 4096
IN = 512
D = 128
H = 4
NCORES = 8
RPC = 1024            # query rows per core
HPC = 2               # heads per core
JCH = N // 128        # 32 source-node chunks
ICH = RPC // 128      # 8 query-row chunks
JG = 8                # j-chunks per group
NG = JCH // JG        # 4 groups
NPAIR = JG // 2       # 4 j-chunk pairs per group
SUPW = 2 * (D + 1)    # per-chunk supp row: [h0|1|h1|1] = 258

_cache = {}


def _build_program():
    import concourse.bacc as bacc
    import concourse.mybir as mybir
    import concourse.tile as tile

    f32 = mybir.dt.float32
    bf16 = mybir.dt.bfloat16
    Exp = mybir.ActivationFunctionType.Exp
    Relu = mybir.ActivationFunctionType.Relu
    add = mybir.AluOpType.add
    mult = mybir.AluOpType.mult
    amax = mybir.AluOpType.max

    n_e = int(os.environ.get("KV2_E", "0"))          # E-path units
    n_r = int(os.environ.get("KV2_R", "13"))           # r-path units
    n_gps = int(os.environ.get("KV2_GPS", "0"))       # gpsimd units
    supp_mode = int(os.environ.get("KV2_SUPP", "2"))  # 0 ACT,1 DVE,2 alt
    NU = NG * NPAIR * HPC  # 32 elementwise units
    e_set = {int((i + 0.5) * NU / n_e) for i in range(n_e)} if n_e else set()
    rest = [u for u in range(NU) if u not in e_set]
    r_set = {rest[int((i + 0.5) * len(rest) / n_r)]
             for i in range(n_r)} if n_r and rest else set()
    rest2 = [u for u in rest if u not in r_set]
    g_set = {rest2[int((i + 0.5) * len(rest2) / n_gps)]
             for i in range(n_gps)} if n_gps and rest2 else set()

    nc = bacc.Bacc(
        "TRN2",
        target_bir_lowering=False,
        debug=False,
        enable_asserts=False,
        num_devices=NCORES,
    )

    adjT = nc.dram_tensor("adjT", [N, RPC], bf16, kind="ExternalInput").ap()
    inpT = nc.dram_tensor("inpT", [IN, N], bf16, kind="ExternalInput").ap()
    wh2 = nc.dram_tensor("wh2", [IN, 2 * D], bf16, kind="ExternalInput").ap()
    whT2 = nc.dram_tensor("whT2", [D, 2 * IN], bf16, kind="ExternalInput").ap()
    uv4 = nc.dram_tensor("uv4", [D, 4], bf16, kind="ExternalInput").ap()
    pwh = nc.dram_tensor("pwh", [IN, 2 * D], bf16, kind="ExternalInput").ap()
    br = nc.dram_tensor("br", [2, 2 * D], f32, kind="ExternalInput").ap()
    outb = nc.dram_tensor("outb", [RPC, 2 * D], f32, kind="ExternalOutput").ap()

    with tile.TileContext(nc) as tc:
        with tc.tile_pool(name="persist", bufs=1) as persist, \
             tc.tile_pool(name="adjp", bufs=3) as adjp, \
             tc.tile_pool(name="pbufp", bufs=6) as pbufp, \
             tc.tile_pool(name="mp", bufs=4) as mp, \
             tc.tile_pool(name="up", bufs=3) as up, \
             tc.tile_pool(name="epp", bufs=2) as epp:
            # ---- persistent tiles ----
            supp_g = [persist.tile([128, JG * SUPW], bf16, tag=f"supp{g}",
                                   name=f"supp{g}") for g in range(NG)]
            f12_g = [persist.tile([128, JG * 4], f32, tag=f"f12{g}",
                                  name=f"f12{g}") for g in range(NG)]
            e1_g = [persist.tile([128, HPC * JG], f32, tag=f"e1{g}",
                                 name=f"e1{g}") for g in range(NG)]
            e1p_g = [persist.tile([128, HPC * JG], f32, tag=f"e1p{g}",
                                  name=f"e1p{g}") for g in range(NG)]
            e1pb_g = [persist.tile([128, HPC * JG], bf16, tag=f"e1pb{g}",
                                   name=f"e1pb{g}") for g in range(NG)]
            f08_g = [persist.tile([128, HPC * JG], f32, tag=f"f08{g}",
                                  name=f"f08{g}") for g in range(NG)]
            f02_g = [persist.tile([128, HPC * JG], f32, tag=f"f02{g}",
                                  name=f"f02{g}") for g in range(NG)]
            F2b = persist.tile([128, HPC * RPC], f32)   # f2 bcast per head
            Gb = persist.tile([128, HPC * RPC], bf16)   # e^{0.8 f2} per head
            res = persist.tile([128, ICH * 2 * D], f32)  # residual+bias
            it = {}
            for kc in range(4):
                for blk in range(4):
                    it[(kc, blk)] = persist.tile(
                        [128, 1024], bf16, tag=f"it{kc}_{blk}",
                        name=f"it{kc}_{blk}")
            rhs_kc = [persist.tile([128, 260], bf16, tag=f"rhs{kc}",
                                   name=f"rhs{kc}") for kc in range(4)]
            pwh_sb = [persist.tile([128, 2 * D], bf16, tag=f"pwh{kc}",
                                   name=f"pwh{kc}") for kc in range(4)]
            whT2_sb = persist.tile([D, 2 * IN], bf16)
            uv4_sb = persist.tile([D, 4], bf16)
            br_sb = persist.tile([1, 4 * D], f32)
            br_row = persist.tile([1, 2 * D], bf16)
            bsum = persist.tile([1, 2 * D], f32)
            ones1 = persist.tile([1, 128], bf16)
            f2row = persist.tile([1, HPC * RPC], f32)

            nc.vector.memset(ones1, 1.0)

            # ---- input DMAs: weights on scalar/gpsimd queues, streams on
            # sync; own-row inputs and group-0 adjacency first ----
            nc.gpsimd.dma_start(out=whT2_sb, in_=whT2)
            nc.gpsimd.dma_start(out=uv4_sb, in_=uv4)
            nc.scalar.dma_start(out=br_sb[0:1, 0:2 * D], in_=br[0:1, :])
            nc.scalar.dma_start(out=br_sb[0:1, 2 * D:4 * D], in_=br[1:2, :])
            for kc in range(4):
                nc.scalar.dma_start(
                    out=rhs_kc[kc][:, 4:260],
                    in_=wh2[kc * 128:(kc + 1) * 128, :])
                nc.scalar.dma_start(
                    out=pwh_sb[kc], in_=pwh[kc * 128:(kc + 1) * 128, :])
            for kc in range(4):
                nc.gpsimd.dma_start(
                    out=it[(kc, 0)], in_=inpT[kc * 128:(kc + 1) * 128, 0:1024])
            for blk in range(1, 4):
                for kc in range(4):
                    nc.gpsimd.dma_start(
                        out=it[(kc, blk)],
                        in_=inpT[kc * 128:(kc + 1) * 128,
                                 blk * 1024:(blk + 1) * 1024])
            # adjacency pair tiles: all on sync queue, in consumption order
            adj_tiles = []
            for g in range(NG):
                for pr in range(NPAIR):
                    jc0 = g * JG + 2 * pr
                    adj_t = adjp.tile([128, 2 * RPC], bf16, tag="adj")
                    for half in range(2):
                        jc = jc0 + half
                        nc.sync.dma_start(
                            out=adj_t[:, half * RPC:(half + 1) * RPC],
                            in_=adjT[jc * 128:(jc + 1) * 128, :])
                    adj_tiles.append(adj_t)

            # supp ones-columns: group 0 first, then broadcasts come early
            nc.gpsimd.memset(supp_g[0], 1.0)

            nc.vector.tensor_add(bsum, br_sb[0:1, 0:2 * D],
                                 br_sb[0:1, 2 * D:4 * D])
            nc.vector.tensor_copy(out=br_row, in_=bsum)

            # ---- early phase: w12, f2 rows, residual ----
            with tc.tile_pool(name="eps", bufs=2, space="PSUM") as eps:
                for h in range(HPC):
                    for kc in range(4):
                        wps = eps.tile([128, 2], f32, tag="w12ps")
                        nc.tensor.matmul(
                            wps,
                            whT2_sb[:, h * IN + kc * 128:h * IN + (kc + 1) * 128],
                            uv4_sb[:, 2 * h:2 * h + 2],
                            start=True, stop=True)
                        nc.vector.tensor_copy(
                            out=rhs_kc[kc][:, 2 * h:2 * h + 2], in_=wps)
                for h in range(HPC):
                    for nh in range(2):
                        f2ps = eps.tile([1, 512], f32, tag="f2ps")
                        for kc in range(4):
                            nc.tensor.matmul(
                                f2ps,
                                rhs_kc[kc][:, 2 * h + 1:2 * h + 2],
                                it[(kc, 0)][:, nh * 512:(nh + 1) * 512],
                                start=(kc == 0), stop=(kc == 3))
                        nc.scalar.copy(
                            out=f2row[0:1, h * RPC + nh * 512:
                                      h * RPC + (nh + 1) * 512],
                            in_=f2ps)


            for h in range(HPC):
                nc.gpsimd.partition_broadcast(
                    F2b[:, h * RPC:(h + 1) * RPC],
                    f2row[0:1, h * RPC:(h + 1) * RPC])
                nc.scalar.activation(
                    Gb[:, h * RPC:(h + 1) * RPC],
                    F2b[:, h * RPC:(h + 1) * RPC], Exp, scale=0.8)
            for g in range(1, NG):
                nc.gpsimd.memset(supp_g[g], 1.0)

            # ---- stage-2 psum accumulators (after eps closes) ----
            acc_tiles = []
            acc_frees = []
            for bk in range(6):
                a_t, a_free = tc.tile([128, 512], f32, space="PSUM",
                                      name=f"acc{bk}")
                acc_tiles.append(a_t)
                acc_frees.append(a_free)

            def acc_slot(h, ic):
                return acc_tiles[h * 3 + ic // 3], (ic % 3) * (D + 1)

            # ---- stage 1 matmuls: ALL emitted first so the PE queue never
            # waits behind stage-2 work; copies trail per group below ----
            with tc.tile_pool(name="s1p", bufs=2, space="PSUM") as s1p:
                ps_list = []
                for nchunk in range(JCH):
                    blk, col = nchunk // 8, nchunk % 8
                    ps = s1p.tile([128, 260], f32, tag="s1ps")
                    for kc in range(4):
                        nc.tensor.matmul(
                            ps,
                            it[(kc, blk)][:, col * 128:(col + 1) * 128],
                            rhs_kc[kc],
                            start=(kc == 0), stop=(kc == 3))
                    ps_list.append(ps)
                    if nchunk == JG - 1:
                        # residual pass after group 0 so it doesn't delay
                        # the group-0 factors on the cold PE; shares the
                        # s1ps psum slots
                        for oc in range(ICH):
                            rps = s1p.tile([128, 260], f32, tag="s1ps")
                            for kc in range(4):
                                nc.tensor.matmul(
                                    rps[:, 0:2 * D],
                                    it[(kc, 0)][:, oc * 128:(oc + 1) * 128],
                                    pwh_sb[kc], start=(kc == 0), stop=False)
                            nc.tensor.matmul(rps[:, 0:2 * D], ones1, br_row,
                                             start=False, stop=True)
                            nc.scalar.copy(
                                out=res[:, oc * 2 * D:(oc + 1) * 2 * D],
                                in_=rps[:, 0:2 * D])

                # ---- per-group: copies + factors + elementwise + agg ----
                uidx = 0
                for g in range(NG):
                    for jo in range(JG):
                        ps = ps_list[g * JG + jo]
                        so = supp_g[g][:, jo * SUPW:(jo + 1) * SUPW].rearrange(
                            "p (h q) -> p h q", q=D + 1)[:, :, 0:D]
                        si = ps[:, 4:260].rearrange("p (h q) -> p h q", q=D)
                        on_dve = (supp_mode == 1 or
                                  (supp_mode == 2 and jo % 2 == 0))
                        if on_dve:
                            nc.vector.tensor_copy(out=so, in_=si)
                        else:
                            nc.scalar.copy(out=so, in_=si)
                        nc.vector.tensor_copy(
                            out=f12_g[g][:, jo * 4:(jo + 1) * 4],
                            in_=ps[:, 0:4])
                    f12v = f12_g[g].rearrange("p (j c) -> p j c", c=4)
                    for h in range(HPC):
                        dst = slice(h * JG, (h + 1) * JG)
                        f1v = f12v[:, :, 2 * h:2 * h + 1]
                        if e_set:
                            nc.scalar.activation(
                                e1pb_g[g][:, dst].rearrange(
                                    "p (j o) -> p j o", o=1), f1v, Exp,
                                scale=0.2)
                        if rest2 or r_set:
                            nc.scalar.activation(
                                e1p_g[g][:, dst].rearrange(
                                    "p (j o) -> p j o", o=1), f1v, Exp,
                                scale=0.2)
                        if rest2:  # d/g paths need e1, Gb
                            nc.scalar.activation(
                                e1_g[g][:, dst].rearrange(
                                    "p (j o) -> p j o", o=1), f1v, Exp)
                        if r_set:
                            nc.vector.tensor_scalar(
                                out=f08_g[g][:, dst].rearrange(
                                    "p (j o) -> p j o", o=1), in0=f1v,
                                scalar1=0.8, scalar2=None, op0=mult)
                            nc.vector.tensor_scalar(
                                out=f02_g[g][:, dst].rearrange(
                                    "p (j o) -> p j o", o=1), in0=f1v,
                                scalar1=0.2, scalar2=None, op0=mult)

                    for pr in range(NPAIR):
                        adj_t = adj_tiles[g * NPAIR + pr]
                        for h in range(HPC):
                            u = uidx
                            uidx += 1
                            p_t = pbufp.tile([128, 2 * RPC], bf16, tag="pbuf")
                            hs = slice(h * RPC, (h + 1) * RPC)
                            if u in e_set:
                                m_t = mp.tile([128, 2 * RPC], bf16, tag="m",
                                              name="m_t")
                                for half in range(2):
                                    jo = 2 * pr + half
                                    sl = slice(half * RPC, (half + 1) * RPC)
                                    col = h * JG + jo
                                    nc.scalar.activation(
                                        m_t[:, sl], F2b[:, hs], Exp,
                                        bias=f12v[:, jo, 2 * h:2 * h + 1],
                                        scale=0.8)
                                    nc.vector.scalar_tensor_tensor(
                                        p_t[:, sl], in0=m_t[:, sl],
                                        scalar=e1pb_g[g][:, col:col + 1],
                                        in1=adj_t[:, sl],
                                        op0=amax, op1=mult)
                            elif u in r_set:
                                for half in range(2):
                                    jo = 2 * pr + half
                                    sl = slice(half * RPC, (half + 1) * RPC)
                                    col = h * JG + jo
                                    # f32 intermediate: bf16 would lose
                                    # ~0.125 abs on args ~27 -> 13% exp err
                                    u_t = up.tile([128, RPC], f32,
                                                  tag="u", name="u_t")
                                    nc.scalar.activation(
                                        u_t, F2b[:, hs], Relu,
                                        bias=f08_g[g][:, col:col + 1],
                                        scale=0.8)
                                    nc.scalar.activation(
                                        p_t[:, sl], u_t, Exp,
                                        bias=f02_g[g][:, col:col + 1])
                                nc.vector.tensor_mul(p_t, adj_t, p_t)
                            elif u in g_set:
                                for half in range(2):
                                    jo = 2 * pr + half
                                    sl = slice(half * RPC, (half + 1) * RPC)
                                    col = h * JG + jo
                                    nc.gpsimd.tensor_scalar(
                                        out=p_t[:, sl], in0=Gb[:, hs],
                                        scalar1=e1_g[g][:, col:col + 1],
                                        scalar2=e1p_g[g][:, col:col + 1],
                                        op0=mult, op1=amax)
                                nc.gpsimd.tensor_mul(p_t, adj_t, p_t)
                            else:
                                for half in range(2):
                                    jo = 2 * pr + half
                                    sl = slice(half * RPC, (half + 1) * RPC)
                                    col = h * JG + jo
                                    nc.vector.tensor_scalar(
                                        out=p_t[:, sl], in0=Gb[:, hs],
                                        scalar1=e1_g[g][:, col:col + 1],
                                        scalar2=e1p_g[g][:, col:col + 1],
                                        op0=mult, op1=amax)
                                nc.vector.tensor_mul(p_t, adj_t, p_t)
                            # aggregation: p-chunks as weights
                            first = (g == 0 and pr == 0)
                            last = (g == NG - 1 and pr == NPAIR - 1)
                            for half in range(2):
                                jo = 2 * pr + half
                                rhs = supp_g[g][:, jo * SUPW + h * (D + 1):
                                                jo * SUPW + (h + 1) * (D + 1)]
                                for ic in range(ICH):
                                    acc, off = acc_slot(h, ic)
                                    lhsT = p_t[:, half * RPC + ic * 128:
                                               half * RPC + (ic + 1) * 128]
                                    nc.tensor.matmul(
                                        acc[:, off:off + D + 1], lhsT, rhs,
                                        start=(first and half == 0
                                               and ic % 3 == 0),
                                        stop=(last and half == 1
                                              and (ic % 3 == 2
                                                   or ic == ICH - 1)),
                                    )

                # ---- epilogue: normalize + residual + store ----
                for h in range(HPC):
                    for b3 in range(3):
                        ics = list(range(b3 * 3, min(b3 * 3 + 3, ICH)))
                        nsl = len(ics)
                        acc = acc_tiles[h * 3 + b3]
                        accv = acc[:, 0:nsl * (D + 1)].rearrange(
                            "p (s q) -> p s q", q=D + 1)
                        den = epp.tile([128, 4], f32, tag="den", name="den")
                        denv = den[:, 0:nsl].rearrange(
                            "p (s o) -> p s o", o=1)
                        nc.vector.tensor_scalar(
                            out=denv, in0=accv[:, :, D:D + 1],
                            scalar1=1e-30, scalar2=None, op0=amax)
                        rc = epp.tile([128, 4], f32, tag="rc")
                        nc.vector.reciprocal(rc[:, 0:nsl], den[:, 0:nsl])
                        for s3, ic in enumerate(ics):
                            of = epp.tile([128, D], f32, tag="of")
                            nc.vector.scalar_tensor_tensor(
                                of,
                                in0=acc[:, s3 * (D + 1):s3 * (D + 1) + D],
                                scalar=rc[:, s3:s3 + 1],
                                in1=res[:, ic * 2 * D + h * D:
                                        ic * 2 * D + (h + 1) * D],
                                op0=mult, op1=add)
                            nc.sync.dma_start(
                                out=outb[ic * 128:(ic + 1) * 128,
                                         h * D:(h + 1) * D],
                                in_=of)
            for a_free in reversed(acc_frees):
                a_free()

    nc.compile()
    return nc


def _get_program():
    key = ("prog3",
           os.environ.get("KV2_E", "0"),
           os.environ.get("KV2_R", "13"),
           os.environ.get("KV2_GPS", "0"),
           os.environ.get("KV2_SUPP", "2"))
    if key not in _cache:
        _cache[key] = _build_program()
    return _cache[key]


def kernel(inputs, adjacency, weight, weight_u, weight_v, bias, proj_w, proj_b):
    from concourse.bass_utils import run_bass_kernel_spmd

    bf = ml_dtypes.bfloat16
    inputs = np.ascontiguousarray(np.asarray(inputs, np.float32))
    adjacency = np.asarray(adjacency, np.float32)
    weight = np.asarray(weight, np.float32)
    weight_u = np.asarray(weight_u, np.float32)
    weight_v = np.asarray(weight_v, np.float32)
    bias = np.asarray(bias, np.float32).reshape(1, H * D)
    proj_w = np.asarray(proj_w, np.float32)
    proj_b = np.asarray(proj_b, np.float32).reshape(H * D)

    nc = _get_program()

    in_maps = []
    for c in range(NCORES):
        h2 = c // 4           # head pair: heads 2*h2, 2*h2+1
        r0 = (c % 4) * RPC
        hs = slice(2 * h2 * D, (2 * h2 + 2) * D)
        rolled = np.roll(inputs, -r0, axis=0)
        inpT_c = np.ascontiguousarray(rolled.T).astype(bf)
        adjT_c = np.ascontiguousarray(
            np.roll(adjacency[r0:r0 + RPC, :], -r0, axis=1).T
        ).astype(bf)  # exact: adjacency is 0.0/1.0
        wh2_c = np.ascontiguousarray(weight[:, hs]).astype(bf)
        whT2_c = np.ascontiguousarray(
            np.concatenate([weight[:, 2 * h2 * D:(2 * h2 + 1) * D].T,
                            weight[:, (2 * h2 + 1) * D:(2 * h2 + 2) * D].T],
                           axis=1)).astype(bf)
        uv4_c = np.ascontiguousarray(
            np.concatenate([weight_u[2 * h2], weight_v[2 * h2],
                            weight_u[2 * h2 + 1], weight_v[2 * h2 + 1]],
                           axis=1)).astype(bf)
        in_maps.append({
            "adjT": adjT_c,
            "inpT": inpT_c,
            "wh2": wh2_c,
            "whT2": whT2_c,
            "uv4": uv4_c,
            "pwh": np.ascontiguousarray(proj_w[:, hs]).astype(bf),
            "br": np.ascontiguousarray(
                np.stack([bias[0, hs], proj_b[hs]], axis=0)),
        })

    trace = os.environ.get("KERNEL_TRACE", "0") == "1"
    results = run_bass_kernel_spmd(
        nc, in_maps, core_ids=list(range(NCORES)), trace=trace)
    _cache["last_results"] = results

    out = np.empty((N, H * D), np.float32)
    for c in range(NCORES):
        h2 = c // 4
        r0 = (c % 4) * RPC
        out[r0:r0 + RPC, 2 * h2 * D:(2 * h2 + 2) * D] = results.results[c]["outb"]
    return out


# revision 16
# speedup vs baseline: 1.0420x; 1.0204x over previous
"""GAT-style dense-mask attention (gnn_message_passing) on 8 trn2 cores.

Sharding v2: core c owns heads {2*(c//4), 2*(c//4)+1} and query rows
[1024*(c%4), +1024).  vs v1 (1 head x 2048 rows) this halves the
adjacency DMA (8.4MB vs 16.8MB/core) and with bf16 inputs halves the
input DMA; every core computes support for ALL nodes for its 2 heads
(no collectives - an all-gather has a ~10us/step latency floor).

Math per core (node order rolled so own rows come first):
  support_h = X @ Wh_h           [4096, 128]  (bf16 operands, f32 psum)
  f1_h = X @ (Wh u), f2_h = X @ (Wh v)
  q_h[j,i] = max(e^{f1[j]} * e^{0.8 f2[i]}, e^{0.2 f1[j]})   (column i
     of the softmax divided by e^{0.2 f2[i]} - cancels in softmax)
  p_h = q_h * adj[i,j]   (exp underflow replaces the -1e30 trick)
  out_h[i,:] = (p_h.T @ [supp_h|1])[:, :128] / denom + X[i] @ proj_w_h
               + bias_h + proj_b_h

Elementwise unit = (j-chunk-pair, head): [128, 2*1024].  Three paths:
  E-path: ACT exp (t = e^{0.8 F2b + f1[j]}) then ONE fused DVE STT
     (t max e1p[j]) * adj per half - all-bf16 operands.
  d-path (pure DVE): dual-op tensor_scalar (Gb *. e1) max. e1p, then
     mask TT.
  g-path: same as d-path but on gpsimd (both ts and mask) to offload.
Emission is two-pass: ALL stage-1 PE matmuls first (PE never waits on
stage 2), then per-group copies + elementwise + aggregation matmuls so
DVE/ACT start group 0 at ~t=8us instead of after the whole stage 1.

Aggregation: p-chunks as PE weights (FWL bf16), rhs = [supp_h|1] 129
cols, persistent psum accumulation across all 32 j-chunks; acc banks
are per-head (matmul start=True zeroes the whole bank).
"""

import os

import ml_dtypes
import numpy as np

N = 4096
IN = 512
D = 128
H = 4
NCORES = 8
RPC = 1024            # query rows per core
HPC = 2               # heads per core
JCH = N // 128        # 32 source-node chunks
ICH = RPC // 128      # 8 query-row chunks
JG = 8                # j-chunks per group
NG = JCH // JG        # 4 groups
NPAIR = JG // 2       # 4 j-chunk pairs per group
SUPW = 2 * (D + 1)    # per-chunk supp row: [h0|1|h1|1] = 258

_cache = {}


def _build_program():
    import concourse.bacc as bacc
    import concourse.mybir as mybir
    import concourse.tile as tile

    f32 = mybir.dt.float32
    bf16 = mybir.dt.bfloat16
    Exp = mybir.ActivationFunctionType.Exp
    Relu = mybir.ActivationFunctionType.Relu
    add = mybir.AluOpType.add
    mult = mybir.AluOpType.mult
    amax = mybir.AluOpType.max

    n_e = int(os.environ.get("KV2_E", "0"))          # E-path units
    n_r = int(os.environ.get("KV2_R", "13"))           # r-path units
    n_gps = int(os.environ.get("KV2_GPS", "0"))       # gpsimd units
    supp_mode = int(os.environ.get("KV2_SUPP", "2"))  # 0 ACT,1 DVE,2 alt
    NU = NG * NPAIR * HPC  # 32 elementwise units
    e_set = {int((i + 0.5) * NU / n_e) for i in range(n_e)} if n_e else set()
    rest = [u for u in range(NU) if u not in e_set]
    r_set = {rest[int((i + 0.5) * len(rest) / n_r)]
             for i in range(n_r)} if n_r and rest else set()
    rest2 = [u for u in rest if u not in r_set]
    g_set = {rest2[int((i + 0.5) * len(rest2) / n_gps)]
             for i in range(n_gps)} if n_gps and rest2 else set()

    nc = bacc.Bacc(
        "TRN2",
        target_bir_lowering=False,
        debug=False,
        enable_asserts=False,
        num_devices=NCORES,
    )

    adjT = nc.dram_tensor("adjT", [N, RPC], bf16, kind="ExternalInput").ap()
    inpT = nc.dram_tensor("inpT", [IN, N], bf16, kind="ExternalInput").ap()
    wh2 = nc.dram_tensor("wh2", [IN, 2 * D], bf16, kind="ExternalInput").ap()
    whT2 = nc.dram_tensor("whT2", [D, 2 * IN], bf16, kind="ExternalInput").ap()
    uv4 = nc.dram_tensor("uv4", [D, 4], bf16, kind="ExternalInput").ap()
    pwh = nc.dram_tensor("pwh", [IN, 2 * D], bf16, kind="ExternalInput").ap()
    br = nc.dram_tensor("br", [2, 2 * D], f32, kind="ExternalInput").ap()
    outb = nc.dram_tensor("outb", [RPC, 2 * D], f32, kind="ExternalOutput").ap()

    with tile.TileContext(nc) as tc:
        with tc.tile_pool(name="persist", bufs=1) as persist, \
             tc.tile_pool(name="adjp", bufs=3) as adjp, \
             tc.tile_pool(name="pbufp", bufs=6) as pbufp, \
             tc.tile_pool(name="mp", bufs=4) as mp, \
             tc.tile_pool(name="up", bufs=3) as up, \
             tc.tile_pool(name="epp", bufs=2) as epp:
            # ---- persistent tiles ----
            supp_g = [persist.tile([128, JG * SUPW], bf16, tag=f"supp{g}",
                                   name=f"supp{g}") for g in range(NG)]
            f12_g = [persist.tile([128, JG * 4], f32, tag=f"f12{g}",
                                  name=f"f12{g}") for g in range(NG)]
            e1_g = [persist.tile([128, HPC * JG], f32, tag=f"e1{g}",
                                 name=f"e1{g}") for g in range(NG)]
            e1p_g = [persist.tile([128, HPC * JG], f32, tag=f"e1p{g}",
                                  name=f"e1p{g}") for g in range(NG)]
            e1pb_g = [persist.tile([128, HPC * JG], bf16, tag=f"e1pb{g}",
                                   name=f"e1pb{g}") for g in range(NG)]
            f08_g = [persist.tile([128, HPC * JG], f32, tag=f"f08{g}",
                                  name=f"f08{g}") for g in range(NG)]
            f02_g = [persist.tile([128, HPC * JG], f32, tag=f"f02{g}",
                                  name=f"f02{g}") for g in range(NG)]
            F2b = persist.tile([128, HPC * RPC], f32)   # f2 bcast per head
            Gb = persist.tile([128, HPC * RPC], bf16)   # e^{0.8 f2} per head
            res = persist.tile([128, ICH * 2 * D], f32)  # residual+bias
            it = {}
            for kc in range(4):
                for blk in range(4):
                    it[(kc, blk)] = persist.tile(
                        [128, 1024], bf16, tag=f"it{kc}_{blk}",
                        name=f"it{kc}_{blk}")
            rhs_kc = [persist.tile([128, 260], bf16, tag=f"rhs{kc}",
                                   name=f"rhs{kc}") for kc in range(4)]
            pwh_sb = [persist.tile([128, 2 * D], bf16, tag=f"pwh{kc}",
                                   name=f"pwh{kc}") for kc in range(4)]
            whT2_sb = persist.tile([D, 2 * IN], bf16)
            uv4_sb = persist.tile([D, 4], bf16)
            br_sb = persist.tile([1, 4 * D], f32)
            br_row = persist.tile([1, 2 * D], bf16)
            bsum = persist.tile([1, 2 * D], f32)
            ones1 = persist.tile([1, 128], bf16)
            f2row = persist.tile([1, HPC * RPC], f32)

            nc.vector.memset(ones1, 1.0)

            # ---- input DMAs: weights on scalar/gpsimd queues, streams on
            # sync; own-row inputs and group-0 adjacency first ----
            nc.gpsimd.dma_start(out=whT2_sb, in_=whT2)
            nc.gpsimd.dma_start(out=uv4_sb, in_=uv4)
            nc.scalar.dma_start(out=br_sb[0:1, 0:2 * D], in_=br[0:1, :])
            nc.scalar.dma_start(out=br_sb[0:1, 2 * D:4 * D], in_=br[1:2, :])
            for kc in range(4):
                nc.scalar.dma_start(
                    out=rhs_kc[kc][:, 4:260],
                    in_=wh2[kc * 128:(kc + 1) * 128, :])
                nc.scalar.dma_start(
                    out=pwh_sb[kc], in_=pwh[kc * 128:(kc + 1) * 128, :])
            # own-row input block on the sync queue BEFORE adjacency so the
            # stage-1 critical path isn't starved by the 8.4MB adj stream
            for kc in range(4):
                nc.sync.dma_start(
                    out=it[(kc, 0)], in_=inpT[kc * 128:(kc + 1) * 128, 0:1024])
            for blk in range(1, 4):
                for kc in range(4):
                    nc.gpsimd.dma_start(
                        out=it[(kc, blk)],
                        in_=inpT[kc * 128:(kc + 1) * 128,
                                 blk * 1024:(blk + 1) * 1024])
            # adjacency pair tiles: all on sync queue, in consumption order
            adj_tiles = []
            for g in range(NG):
                for pr in range(NPAIR):
                    jc0 = g * JG + 2 * pr
                    adj_t = adjp.tile([128, 2 * RPC], bf16, tag="adj")
                    for half in range(2):
                        jc = jc0 + half
                        nc.sync.dma_start(
                            out=adj_t[:, half * RPC:(half + 1) * RPC],
                            in_=adjT[jc * 128:(jc + 1) * 128, :])
                    adj_tiles.append(adj_t)

            # supp ones-columns: group 0 first, then broadcasts come early
            nc.gpsimd.memset(supp_g[0], 1.0)

            nc.vector.tensor_add(bsum, br_sb[0:1, 0:2 * D],
                                 br_sb[0:1, 2 * D:4 * D])
            nc.vector.tensor_copy(out=br_row, in_=bsum)

            # ---- early phase: w12, f2 rows, residual ----
            with tc.tile_pool(name="eps", bufs=2, space="PSUM") as eps:
                for h in range(HPC):
                    for kc in range(4):
                        wps = eps.tile([128, 2], f32, tag="w12ps")
                        nc.tensor.matmul(
                            wps,
                            whT2_sb[:, h * IN + kc * 128:h * IN + (kc + 1) * 128],
                            uv4_sb[:, 2 * h:2 * h + 2],
                            start=True, stop=True)
                        nc.vector.tensor_copy(
                            out=rhs_kc[kc][:, 2 * h:2 * h + 2], in_=wps)
                for h in range(HPC):
                    for nh in range(2):
                        f2ps = eps.tile([1, 512], f32, tag="f2ps")
                        for kc in range(4):
                            nc.tensor.matmul(
                                f2ps,
                                rhs_kc[kc][:, 2 * h + 1:2 * h + 2],
                                it[(kc, 0)][:, nh * 512:(nh + 1) * 512],
                                start=(kc == 0), stop=(kc == 3))
                        nc.scalar.copy(
                            out=f2row[0:1, h * RPC + nh * 512:
                                      h * RPC + (nh + 1) * 512],
                            in_=f2ps)


            for h in range(HPC):
                nc.gpsimd.partition_broadcast(
                    F2b[:, h * RPC:(h + 1) * RPC],
                    f2row[0:1, h * RPC:(h + 1) * RPC])
                nc.scalar.activation(
                    Gb[:, h * RPC:(h + 1) * RPC],
                    F2b[:, h * RPC:(h + 1) * RPC], Exp, scale=0.8)
            for g in range(1, NG):
                nc.gpsimd.memset(supp_g[g], 1.0)

            # ---- stage-2 psum accumulators (after eps closes) ----
            acc_tiles = []
            acc_frees = []
            for bk in range(6):
                a_t, a_free = tc.tile([128, 512], f32, space="PSUM",
                                      name=f"acc{bk}")
                acc_tiles.append(a_t)
                acc_frees.append(a_free)

            def acc_slot(h, ic):
                return acc_tiles[h * 3 + ic // 3], (ic % 3) * (D + 1)

            # ---- stage 1 matmuls: ALL emitted first so the PE queue never
            # waits behind stage-2 work; copies trail per group below ----
            with tc.tile_pool(name="s1p", bufs=2, space="PSUM") as s1p:
                ps_list = []
                for nchunk in range(JCH):
                    blk, col = nchunk // 8, nchunk % 8
                    ps = s1p.tile([128, 260], f32, tag="s1ps")
                    for kc in range(4):
                        nc.tensor.matmul(
                            ps,
                            it[(kc, blk)][:, col * 128:(col + 1) * 128],
                            rhs_kc[kc],
                            start=(kc == 0), stop=(kc == 3))
                    ps_list.append(ps)
                    if nchunk == JG - 1:
                        # residual pass after group 0 so it doesn't delay
                        # the group-0 factors on the cold PE; shares the
                        # s1ps psum slots
                        for oc in range(ICH):
                            rps = s1p.tile([128, 260], f32, tag="s1ps")
                            for kc in range(4):
                                nc.tensor.matmul(
                                    rps[:, 0:2 * D],
                                    it[(kc, 0)][:, oc * 128:(oc + 1) * 128],
                                    pwh_sb[kc], start=(kc == 0), stop=False)
                            nc.tensor.matmul(rps[:, 0:2 * D], ones1, br_row,
                                             start=False, stop=True)
                            nc.scalar.copy(
                                out=res[:, oc * 2 * D:(oc + 1) * 2 * D],
                                in_=rps[:, 0:2 * D])

                # ---- per-group: copies + factors + elementwise + agg ----
                uidx = 0
                for g in range(NG):
                    for jo in range(JG):
                        ps = ps_list[g * JG + jo]
                        so = supp_g[g][:, jo * SUPW:(jo + 1) * SUPW].rearrange(
                            "p (h q) -> p h q", q=D + 1)[:, :, 0:D]
                        si = ps[:, 4:260].rearrange("p (h q) -> p h q", q=D)
                        on_dve = (supp_mode == 1 or
                                  (supp_mode == 2 and jo % 2 == 0))
                        if on_dve:
                            nc.vector.tensor_copy(out=so, in_=si)
                        else:
                            nc.scalar.copy(out=so, in_=si)
                        nc.vector.tensor_copy(
                            out=f12_g[g][:, jo * 4:(jo + 1) * 4],
                            in_=ps[:, 0:4])
                    f12v = f12_g[g].rearrange("p (j c) -> p j c", c=4)
                    for h in range(HPC):
                        dst = slice(h * JG, (h + 1) * JG)
                        f1v = f12v[:, :, 2 * h:2 * h + 1]
                        if e_set:
                            nc.scalar.activation(
                                e1pb_g[g][:, dst].rearrange(
                                    "p (j o) -> p j o", o=1), f1v, Exp,
                                scale=0.2)
                        if rest2 or r_set:
                            nc.scalar.activation(
                                e1p_g[g][:, dst].rearrange(
                                    "p (j o) -> p j o", o=1), f1v, Exp,
                                scale=0.2)
                        if rest2:  # d/g paths need e1, Gb
                            nc.scalar.activation(
                                e1_g[g][:, dst].rearrange(
                                    "p (j o) -> p j o", o=1), f1v, Exp)
                        if r_set:
                            nc.vector.tensor_scalar(
                                out=f08_g[g][:, dst].rearrange(
                                    "p (j o) -> p j o", o=1), in0=f1v,
                                scalar1=0.8, scalar2=None, op0=mult)
                            nc.vector.tensor_scalar(
                                out=f02_g[g][:, dst].rearrange(
                                    "p (j o) -> p j o", o=1), in0=f1v,
                                scalar1=0.2, scalar2=None, op0=mult)

                    for pr in range(NPAIR):
                        adj_t = adj_tiles[g * NPAIR + pr]
                        for h in range(HPC):
                            u = uidx
                            uidx += 1
                            p_t = pbufp.tile([128, 2 * RPC], bf16, tag="pbuf")
                            hs = slice(h * RPC, (h + 1) * RPC)
                            if u in e_set:
                                m_t = mp.tile([128, 2 * RPC], bf16, tag="m",
                                              name="m_t")
                                for half in range(2):
                                    jo = 2 * pr + half
                                    sl = slice(half * RPC, (half + 1) * RPC)
                                    col = h * JG + jo
                                    nc.scalar.activation(
                                        m_t[:, sl], F2b[:, hs], Exp,
                                        bias=f12v[:, jo, 2 * h:2 * h + 1],
                                        scale=0.8)
                                    nc.vector.scalar_tensor_tensor(
                                        p_t[:, sl], in0=m_t[:, sl],
                                        scalar=e1pb_g[g][:, col:col + 1],
                                        in1=adj_t[:, sl],
                                        op0=amax, op1=mult)
                            elif u in r_set:
                                for half in range(2):
                                    jo = 2 * pr + half
                                    sl = slice(half * RPC, (half + 1) * RPC)
                                    col = h * JG + jo
                                    # f32 intermediate: bf16 would lose
                                    # ~0.125 abs on args ~27 -> 13% exp err
                                    u_t = up.tile([128, RPC], f32,
                                                  tag="u", name="u_t")
                                    nc.scalar.activation(
                                        u_t, F2b[:, hs], Relu,
                                        bias=f08_g[g][:, col:col + 1],
                                        scale=0.8)
                                    nc.scalar.activation(
                                        p_t[:, sl], u_t, Exp,
                                        bias=f02_g[g][:, col:col + 1])
                                nc.vector.tensor_mul(p_t, adj_t, p_t)
                            elif u in g_set:
                                for half in range(2):
                                    jo = 2 * pr + half
                                    sl = slice(half * RPC, (half + 1) * RPC)
                                    col = h * JG + jo
                                    nc.gpsimd.tensor_scalar(
                                        out=p_t[:, sl], in0=Gb[:, hs],
                                        scalar1=e1_g[g][:, col:col + 1],
                                        scalar2=e1p_g[g][:, col:col + 1],
                                        op0=mult, op1=amax)
                                nc.gpsimd.tensor_mul(p_t, adj_t, p_t)
                            else:
                                for half in range(2):
                                    jo = 2 * pr + half
                                    sl = slice(half * RPC, (half + 1) * RPC)
                                    col = h * JG + jo
                                    nc.vector.tensor_scalar(
                                        out=p_t[:, sl], in0=Gb[:, hs],
                                        scalar1=e1_g[g][:, col:col + 1],
                                        scalar2=e1p_g[g][:, col:col + 1],
                                        op0=mult, op1=amax)
                                nc.vector.tensor_mul(p_t, adj_t, p_t)
                            # aggregation: p-chunks as weights
                            first = (g == 0 and pr == 0)
                            last = (g == NG - 1 and pr == NPAIR - 1)
                            for half in range(2):
                                jo = 2 * pr + half
                                rhs = supp_g[g][:, jo * SUPW + h * (D + 1):
                                                jo * SUPW + (h + 1) * (D + 1)]
                                for ic in range(ICH):
                                    acc, off = acc_slot(h, ic)
                                    lhsT = p_t[:, half * RPC + ic * 128:
                                               half * RPC + (ic + 1) * 128]
                                    nc.tensor.matmul(
                                        acc[:, off:off + D + 1], lhsT, rhs,
                                        start=(first and half == 0
                                               and ic % 3 == 0),
                                        stop=(last and half == 1
                                              and (ic % 3 == 2
                                                   or ic == ICH - 1)),
                                    )

                # ---- epilogue: normalize + residual + store ----
                for h in range(HPC):
                    for b3 in range(3):
                        ics = list(range(b3 * 3, min(b3 * 3 + 3, ICH)))
                        nsl = len(ics)
                        acc = acc_tiles[h * 3 + b3]
                        accv = acc[:, 0:nsl * (D + 1)].rearrange(
                            "p (s q) -> p s q", q=D + 1)
                        den = epp.tile([128, 4], f32, tag="den", name="den")
                        denv = den[:, 0:nsl].rearrange(
                            "p (s o) -> p s o", o=1)
                        nc.vector.tensor_scalar(
                            out=denv, in0=accv[:, :, D:D + 1],
                            scalar1=1e-30, scalar2=None, op0=amax)
                        rc = epp.tile([128, 4], f32, tag="rc")
                        nc.vector.reciprocal(rc[:, 0:nsl], den[:, 0:nsl])
                        for s3, ic in enumerate(ics):
                            of = epp.tile([128, D], f32, tag="of")
                            nc.vector.scalar_tensor_tensor(
                                of,
                                in0=acc[:, s3 * (D + 1):s3 * (D + 1) + D],
                                scalar=rc[:, s3:s3 + 1],
                                in1=res[:, ic * 2 * D + h * D:
                                        ic * 2 * D + (h + 1) * D],
                                op0=mult, op1=add)
                            nc.sync.dma_start(
                                out=outb[ic * 128:(ic + 1) * 128,
                                         h * D:(h + 1) * D],
                                in_=of)
            for a_free in reversed(acc_frees):
                a_free()

    nc.compile()
    return nc


def _get_program():
    key = ("prog3",
           os.environ.get("KV2_E", "0"),
           os.environ.get("KV2_R", "13"),
           os.environ.get("KV2_GPS", "0"),
           os.environ.get("KV2_SUPP", "2"))
    if key not in _cache:
        _cache[key] = _build_program()
    return _cache[key]


def kernel(inputs, adjacency, weight, weight_u, weight_v, bias, proj_w, proj_b):
    from concourse.bass_utils import run_bass_kernel_spmd

    bf = ml_dtypes.bfloat16
    inputs = np.ascontiguousarray(np.asarray(inputs, np.float32))
    adjacency = np.asarray(adjacency, np.float32)
    weight = np.asarray(weight, np.float32)
    weight_u = np.asarray(weight_u, np.float32)
    weight_v = np.asarray(weight_v, np.float32)
    bias = np.asarray(bias, np.float32).reshape(1, H * D)
    proj_w = np.asarray(proj_w, np.float32)
    proj_b = np.asarray(proj_b, np.float32).reshape(H * D)

    nc = _get_program()

    in_maps = []
    for c in range(NCORES):
        h2 = c // 4           # head pair: heads 2*h2, 2*h2+1
        r0 = (c % 4) * RPC
        hs = slice(2 * h2 * D, (2 * h2 + 2) * D)
        rolled = np.roll(inputs, -r0, axis=0)
        inpT_c = np.ascontiguousarray(rolled.T).astype(bf)
        adjT_c = np.ascontiguousarray(
            np.roll(adjacency[r0:r0 + RPC, :], -r0, axis=1).T
        ).astype(bf)  # exact: adjacency is 0.0/1.0
        wh2_c = np.ascontiguousarray(weight[:, hs]).astype(bf)
        whT2_c = np.ascontiguousarray(
            np.concatenate([weight[:, 2 * h2 * D:(2 * h2 + 1) * D].T,
                            weight[:, (2 * h2 + 1) * D:(2 * h2 + 2) * D].T],
                           axis=1)).astype(bf)
        uv4_c = np.ascontiguousarray(
            np.concatenate([weight_u[2 * h2], weight_v[2 * h2],
                            weight_u[2 * h2 + 1], weight_v[2 * h2 + 1]],
                           axis=1)).astype(bf)
        in_maps.append({
            "adjT": adjT_c,
            "inpT": inpT_c,
            "wh2": wh2_c,
            "whT2": whT2_c,
            "uv4": uv4_c,
            "pwh": np.ascontiguousarray(proj_w[:, hs]).astype(bf),
            "br": np.ascontiguousarray(
                np.stack([bias[0, hs], proj_b[hs]], axis=0)),
        })

    trace = os.environ.get("KERNEL_TRACE", "0") == "1"
    results = run_bass_kernel_spmd(
        nc, in_maps, core_ids=list(range(NCORES)), trace=trace)
    _cache["last_results"] = results

    out = np.empty((N, H * D), np.float32)
    for c in range(NCORES):
        h2 = c // 4
        r0 = (c % 4) * RPC
        out[r0:r0 + RPC, 2 * h2 * D:(2 * h2 + 2) * D] = results.results[c]["outb"]
    return out


# revision 19
# speedup vs baseline: 1.0486x; 1.0063x over previous
"""GAT-style dense-mask attention (gnn_message_passing) on 8 trn2 cores.

Sharding v2: core c owns heads {2*(c//4), 2*(c//4)+1} and query rows
[1024*(c%4), +1024).  vs v1 (1 head x 2048 rows) this halves the
adjacency DMA (8.4MB vs 16.8MB/core) and with bf16 inputs halves the
input DMA; every core computes support for ALL nodes for its 2 heads
(no collectives - an all-gather has a ~10us/step latency floor).

Math per core (node order rolled so own rows come first):
  support_h = X @ Wh_h           [4096, 128]  (bf16 operands, f32 psum)
  f1_h = X @ (Wh u), f2_h = X @ (Wh v)
  q_h[j,i] = max(e^{f1[j]} * e^{0.8 f2[i]}, e^{0.2 f1[j]})   (column i
     of the softmax divided by e^{0.2 f2[i]} - cancels in softmax)
  p_h = q_h * adj[i,j]   (exp underflow replaces the -1e30 trick)
  out_h[i,:] = (p_h.T @ [supp_h|1])[:, :128] / denom + X[i] @ proj_w_h
               + bias_h + proj_b_h

Elementwise unit = (j-chunk-pair, head): [128, 2*1024].  Three paths:
  E-path: ACT exp (t = e^{0.8 F2b + f1[j]}) then ONE fused DVE STT
     (t max e1p[j]) * adj per half - all-bf16 operands.
  d-path (pure DVE): dual-op tensor_scalar (Gb *. e1) max. e1p, then
     mask TT.
  g-path: same as d-path but on gpsimd (both ts and mask) to offload.
Emission is two-pass: ALL stage-1 PE matmuls first (PE never waits on
stage 2), then per-group copies + elementwise + aggregation matmuls so
DVE/ACT start group 0 at ~t=8us instead of after the whole stage 1.

Aggregation: p-chunks as PE weights (FWL bf16), rhs = [supp_h|1] 129
cols, persistent psum accumulation across all 32 j-chunks; acc banks
are per-head (matmul start=True zeroes the whole bank).
"""

import os

import ml_dtypes
import numpy as np

N = 4096
IN = 512
D = 128
H = 4
NCORES = 8
RPC = 1024            # query rows per core
HPC = 2               # heads per core
JCH = N // 128        # 32 source-node chunks
ICH = RPC // 128      # 8 query-row chunks
JG = 8                # j-chunks per group
NG = JCH // JG        # 4 groups
NPAIR = JG // 2       # 4 j-chunk pairs per group
SUPW = 2 * (D + 1)    # per-chunk supp row: [h0|1|h1|1] = 258

_cache = {}


def _build_program():
    import concourse.bacc as bacc
    import concourse.mybir as mybir
    import concourse.tile as tile

    f32 = mybir.dt.float32
    bf16 = mybir.dt.bfloat16
    Exp = mybir.ActivationFunctionType.Exp
    Relu = mybir.ActivationFunctionType.Relu
    add = mybir.AluOpType.add
    mult = mybir.AluOpType.mult
    amax = mybir.AluOpType.max

    n_e = int(os.environ.get("KV2_E", "0"))          # E-path units
    n_r = int(os.environ.get("KV2_R", "13"))           # r-path units
    n_gps = int(os.environ.get("KV2_GPS", "0"))       # gpsimd units
    supp_mode = int(os.environ.get("KV2_SUPP", "2"))  # 0 ACT,1 DVE,2 alt
    NU = NG * NPAIR * HPC  # 32 elementwise units
    e_set = {int((i + 0.5) * NU / n_e) for i in range(n_e)} if n_e else set()
    rest = [u for u in range(NU) if u not in e_set]
    r_set = {rest[int((i + 0.5) * len(rest) / n_r)]
             for i in range(n_r)} if n_r and rest else set()
    rest2 = [u for u in rest if u not in r_set]
    g_set = {rest2[int((i + 0.5) * len(rest2) / n_gps)]
             for i in range(n_gps)} if n_gps and rest2 else set()

    nc = bacc.Bacc(
        "TRN2",
        target_bir_lowering=False,
        debug=False,
        enable_asserts=False,
        num_devices=NCORES,
    )

    adjT = nc.dram_tensor("adjT", [N, RPC], bf16, kind="ExternalInput").ap()
    inpT = nc.dram_tensor("inpT", [IN, N], bf16, kind="ExternalInput").ap()
    wh2 = nc.dram_tensor("wh2", [IN, 2 * D], bf16, kind="ExternalInput").ap()
    whT2 = nc.dram_tensor("whT2", [D, 2 * IN], bf16, kind="ExternalInput").ap()
    uv4 = nc.dram_tensor("uv4", [D, 4], bf16, kind="ExternalInput").ap()
    pwh = nc.dram_tensor("pwh", [IN, 2 * D], bf16, kind="ExternalInput").ap()
    br = nc.dram_tensor("br", [2, 2 * D], f32, kind="ExternalInput").ap()
    outb = nc.dram_tensor("outb", [RPC, 2 * D], f32, kind="ExternalOutput").ap()

    with tile.TileContext(nc) as tc:
        with tc.tile_pool(name="persist", bufs=1) as persist, \
             tc.tile_pool(name="adjp", bufs=4) as adjp, \
             tc.tile_pool(name="pbufp", bufs=7) as pbufp, \
             tc.tile_pool(name="mp", bufs=4) as mp, \
             tc.tile_pool(name="up", bufs=4) as up, \
             tc.tile_pool(name="epp", bufs=2) as epp:
            # ---- persistent tiles ----
            supp_g = [persist.tile([128, JG * SUPW], bf16, tag=f"supp{g}",
                                   name=f"supp{g}") for g in range(NG)]
            f12_g = [persist.tile([128, JG * 4], f32, tag=f"f12{g}",
                                  name=f"f12{g}") for g in range(NG)]
            e1_g = [persist.tile([128, HPC * JG], f32, tag=f"e1{g}",
                                 name=f"e1{g}") for g in range(NG)]
            e1p_g = [persist.tile([128, HPC * JG], f32, tag=f"e1p{g}",
                                  name=f"e1p{g}") for g in range(NG)]
            e1pb_g = [persist.tile([128, HPC * JG], bf16, tag=f"e1pb{g}",
                                   name=f"e1pb{g}") for g in range(NG)]
            f08_g = [persist.tile([128, HPC * JG], f32, tag=f"f08{g}",
                                  name=f"f08{g}") for g in range(NG)]
            f02_g = [persist.tile([128, HPC * JG], f32, tag=f"f02{g}",
                                  name=f"f02{g}") for g in range(NG)]
            F2b = persist.tile([128, HPC * RPC], f32)   # f2 bcast per head
            Gb = persist.tile([128, HPC * RPC], bf16)   # e^{0.8 f2} per head
            res = persist.tile([128, ICH * 2 * D], f32)  # residual+bias
            it = {}
            for kc in range(4):
                for blk in range(4):
                    it[(kc, blk)] = persist.tile(
                        [128, 1024], bf16, tag=f"it{kc}_{blk}",
                        name=f"it{kc}_{blk}")
            rhs_kc = [persist.tile([128, 260], bf16, tag=f"rhs{kc}",
                                   name=f"rhs{kc}") for kc in range(4)]
            pwh_sb = [persist.tile([128, 2 * D], bf16, tag=f"pwh{kc}",
                                   name=f"pwh{kc}") for kc in range(4)]
            whT2_sb = persist.tile([D, 2 * IN], bf16)
            uv4_sb = persist.tile([D, 4], bf16)
            br_sb = persist.tile([1, 4 * D], f32)
            br_row = persist.tile([1, 2 * D], bf16)
            bsum = persist.tile([1, 2 * D], f32)
            ones1 = persist.tile([1, 128], bf16)
            ones512 = persist.tile([1, 512], bf16)
            f2row = persist.tile([1, HPC * RPC], f32)

            nc.vector.memset(ones1, 1.0)
            nc.vector.memset(ones512, 1.0)

            # ---- input DMAs: weights on scalar/gpsimd queues, streams on
            # sync; own-row inputs and group-0 adjacency first ----
            nc.gpsimd.dma_start(out=whT2_sb, in_=whT2)
            nc.gpsimd.dma_start(out=uv4_sb, in_=uv4)
            nc.scalar.dma_start(out=br_sb[0:1, 0:2 * D], in_=br[0:1, :])
            nc.scalar.dma_start(out=br_sb[0:1, 2 * D:4 * D], in_=br[1:2, :])
            for kc in range(4):
                nc.scalar.dma_start(
                    out=rhs_kc[kc][:, 4:260],
                    in_=wh2[kc * 128:(kc + 1) * 128, :])
                nc.scalar.dma_start(
                    out=pwh_sb[kc], in_=pwh[kc * 128:(kc + 1) * 128, :])
            # own-row input block on the sync queue BEFORE adjacency so the
            # stage-1 critical path isn't starved by the 8.4MB adj stream
            for kc in range(4):
                nc.sync.dma_start(
                    out=it[(kc, 0)], in_=inpT[kc * 128:(kc + 1) * 128, 0:1024])
            for blk in range(1, 4):
                for kc in range(4):
                    nc.gpsimd.dma_start(
                        out=it[(kc, blk)],
                        in_=inpT[kc * 128:(kc + 1) * 128,
                                 blk * 1024:(blk + 1) * 1024])
            # adjacency pair tiles: all on sync queue, in consumption order
            adj_tiles = []
            for g in range(NG):
                for pr in range(NPAIR):
                    jc0 = g * JG + 2 * pr
                    adj_t = adjp.tile([128, 2 * RPC], bf16, tag="adj")
                    for half in range(2):
                        jc = jc0 + half
                        nc.sync.dma_start(
                            out=adj_t[:, half * RPC:(half + 1) * RPC],
                            in_=adjT[jc * 128:(jc + 1) * 128, :])
                    adj_tiles.append(adj_t)

            # supp ones-columns: group 0 first, then broadcasts come early
            nc.gpsimd.memset(supp_g[0], 1.0)

            nc.vector.tensor_add(bsum, br_sb[0:1, 0:2 * D],
                                 br_sb[0:1, 2 * D:4 * D])
            nc.vector.tensor_copy(out=br_row, in_=bsum)

            # ---- early phase: w12, f2 rows, residual ----
            with tc.tile_pool(name="eps", bufs=2, space="PSUM") as eps:
                # PE warm-up: the HAM clock gate needs ~3.4us of sustained
                # activity to switch 1.2 -> 2.4 GHz.  Burn it on dummy
                # matmuls over the ones tiles (ready at ~t=0.3us) so the
                # real w12 -> f2 -> stage-1 critical chain runs warm.
                for i in range(12):
                    wmps = eps.tile([128, 512], f32, tag="warmps")
                    nc.tensor.matmul(wmps, ones1, ones512,
                                     start=True, stop=True)
                for h in range(HPC):
                    for kc in range(4):
                        wps = eps.tile([128, 2], f32, tag="w12ps")
                        nc.tensor.matmul(
                            wps,
                            whT2_sb[:, h * IN + kc * 128:h * IN + (kc + 1) * 128],
                            uv4_sb[:, 2 * h:2 * h + 2],
                            start=True, stop=True)
                        nc.vector.tensor_copy(
                            out=rhs_kc[kc][:, 2 * h:2 * h + 2], in_=wps)
                for h in range(HPC):
                    for nh in range(2):
                        f2ps = eps.tile([1, 512], f32, tag="f2ps")
                        for kc in range(4):
                            nc.tensor.matmul(
                                f2ps,
                                rhs_kc[kc][:, 2 * h + 1:2 * h + 2],
                                it[(kc, 0)][:, nh * 512:(nh + 1) * 512],
                                start=(kc == 0), stop=(kc == 3))
                        nc.scalar.copy(
                            out=f2row[0:1, h * RPC + nh * 512:
                                      h * RPC + (nh + 1) * 512],
                            in_=f2ps)


            for h in range(HPC):
                nc.gpsimd.partition_broadcast(
                    F2b[:, h * RPC:(h + 1) * RPC],
                    f2row[0:1, h * RPC:(h + 1) * RPC])
                nc.scalar.activation(
                    Gb[:, h * RPC:(h + 1) * RPC],
                    F2b[:, h * RPC:(h + 1) * RPC], Exp, scale=0.8)
            for g in range(1, NG):
                nc.gpsimd.memset(supp_g[g], 1.0)

            # ---- stage-2 psum accumulators (after eps closes) ----
            acc_tiles = []
            acc_frees = []
            for bk in range(6):
                a_t, a_free = tc.tile([128, 512], f32, space="PSUM",
                                      name=f"acc{bk}")
                acc_tiles.append(a_t)
                acc_frees.append(a_free)

            def acc_slot(h, ic):
                return acc_tiles[h * 3 + ic // 3], (ic % 3) * (D + 1)

            # ---- stage 1 matmuls: ALL emitted first so the PE queue never
            # waits behind stage-2 work; copies trail per group below ----
            with tc.tile_pool(name="s1p", bufs=2, space="PSUM") as s1p:
                ps_list = []
                for nchunk in range(JCH):
                    blk, col = nchunk // 8, nchunk % 8
                    ps = s1p.tile([128, 260], f32, tag="s1ps")
                    for kc in range(4):
                        nc.tensor.matmul(
                            ps,
                            it[(kc, blk)][:, col * 128:(col + 1) * 128],
                            rhs_kc[kc],
                            start=(kc == 0), stop=(kc == 3))
                    ps_list.append(ps)
                    if nchunk == JG - 1:
                        # residual pass after group 0 so it doesn't delay
                        # the group-0 factors on the cold PE; shares the
                        # s1ps psum slots
                        for oc in range(ICH):
                            rps = s1p.tile([128, 260], f32, tag="s1ps")
                            for kc in range(4):
                                nc.tensor.matmul(
                                    rps[:, 0:2 * D],
                                    it[(kc, 0)][:, oc * 128:(oc + 1) * 128],
                                    pwh_sb[kc], start=(kc == 0), stop=False)
                            nc.tensor.matmul(rps[:, 0:2 * D], ones1, br_row,
                                             start=False, stop=True)
                            nc.scalar.copy(
                                out=res[:, oc * 2 * D:(oc + 1) * 2 * D],
                                in_=rps[:, 0:2 * D])

                # ---- per-group: copies + factors + elementwise + agg ----
                uidx = 0
                for g in range(NG):
                    for jo in range(JG):
                        ps = ps_list[g * JG + jo]
                        so = supp_g[g][:, jo * SUPW:(jo + 1) * SUPW].rearrange(
                            "p (h q) -> p h q", q=D + 1)[:, :, 0:D]
                        si = ps[:, 4:260].rearrange("p (h q) -> p h q", q=D)
                        on_dve = (supp_mode == 1 or
                                  (supp_mode == 2 and jo % 2 == 0))
                        if on_dve:
                            nc.vector.tensor_copy(out=so, in_=si)
                        else:
                            nc.scalar.copy(out=so, in_=si)
                        nc.vector.tensor_copy(
                            out=f12_g[g][:, jo * 4:(jo + 1) * 4],
                            in_=ps[:, 0:4])
                    f12v = f12_g[g].rearrange("p (j c) -> p j c", c=4)
                    for h in range(HPC):
                        dst = slice(h * JG, (h + 1) * JG)
                        f1v = f12v[:, :, 2 * h:2 * h + 1]
                        if e_set:
                            nc.scalar.activation(
                                e1pb_g[g][:, dst].rearrange(
                                    "p (j o) -> p j o", o=1), f1v, Exp,
                                scale=0.2)
                        if rest2 or r_set:
                            nc.scalar.activation(
                                e1p_g[g][:, dst].rearrange(
                                    "p (j o) -> p j o", o=1), f1v, Exp,
                                scale=0.2)
                        if rest2:  # d/g paths need e1, Gb
                            nc.scalar.activation(
                                e1_g[g][:, dst].rearrange(
                                    "p (j o) -> p j o", o=1), f1v, Exp)
                        if r_set:
                            nc.vector.tensor_scalar(
                                out=f08_g[g][:, dst].rearrange(
                                    "p (j o) -> p j o", o=1), in0=f1v,
                                scalar1=0.8, scalar2=None, op0=mult)
                            nc.vector.tensor_scalar(
                                out=f02_g[g][:, dst].rearrange(
                                    "p (j o) -> p j o", o=1), in0=f1v,
                                scalar1=0.2, scalar2=None, op0=mult)

                    for pr in range(NPAIR):
                        adj_t = adj_tiles[g * NPAIR + pr]
                        for h in range(HPC):
                            u = uidx
                            uidx += 1
                            p_t = pbufp.tile([128, 2 * RPC], bf16, tag="pbuf")
                            hs = slice(h * RPC, (h + 1) * RPC)
                            if u in e_set:
                                m_t = mp.tile([128, 2 * RPC], bf16, tag="m",
                                              name="m_t")
                                for half in range(2):
                                    jo = 2 * pr + half
                                    sl = slice(half * RPC, (half + 1) * RPC)
                                    col = h * JG + jo
                                    nc.scalar.activation(
                                        m_t[:, sl], F2b[:, hs], Exp,
                                        bias=f12v[:, jo, 2 * h:2 * h + 1],
                                        scale=0.8)
                                    nc.vector.scalar_tensor_tensor(
                                        p_t[:, sl], in0=m_t[:, sl],
                                        scalar=e1pb_g[g][:, col:col + 1],
                                        in1=adj_t[:, sl],
                                        op0=amax, op1=mult)
                            elif u in r_set:
                                for half in range(2):
                                    jo = 2 * pr + half
                                    sl = slice(half * RPC, (half + 1) * RPC)
                                    col = h * JG + jo
                                    # f32 intermediate: bf16 would lose
                                    # ~0.125 abs on args ~27 -> 13% exp err
                                    u_t = up.tile([128, RPC], f32,
                                                  tag="u", name="u_t")
                                    nc.scalar.activation(
                                        u_t, F2b[:, hs], Relu,
                                        bias=f08_g[g][:, col:col + 1],
                                        scale=0.8)
                                    nc.scalar.activation(
                                        p_t[:, sl], u_t, Exp,
                                        bias=f02_g[g][:, col:col + 1])
                                nc.vector.tensor_mul(p_t, adj_t, p_t)
                            elif u in g_set:
                                for half in range(2):
                                    jo = 2 * pr + half
                                    sl = slice(half * RPC, (half + 1) * RPC)
                                    col = h * JG + jo
                                    nc.gpsimd.tensor_scalar(
                                        out=p_t[:, sl], in0=Gb[:, hs],
                                        scalar1=e1_g[g][:, col:col + 1],
                                        scalar2=e1p_g[g][:, col:col + 1],
                                        op0=mult, op1=amax)
                                nc.gpsimd.tensor_mul(p_t, adj_t, p_t)
                            else:
                                for half in range(2):
                                    jo = 2 * pr + half
                                    sl = slice(half * RPC, (half + 1) * RPC)
                                    col = h * JG + jo
                                    nc.vector.tensor_scalar(
                                        out=p_t[:, sl], in0=Gb[:, hs],
                                        scalar1=e1_g[g][:, col:col + 1],
                                        scalar2=e1p_g[g][:, col:col + 1],
                                        op0=mult, op1=amax)
                                nc.vector.tensor_mul(p_t, adj_t, p_t)
                            # aggregation: p-chunks as weights
                            first = (g == 0 and pr == 0)
                            last = (g == NG - 1 and pr == NPAIR - 1)
                            for half in range(2):
                                jo = 2 * pr + half
                                rhs = supp_g[g][:, jo * SUPW + h * (D + 1):
                                                jo * SUPW + (h + 1) * (D + 1)]
                                for ic in range(ICH):
                                    acc, off = acc_slot(h, ic)
                                    lhsT = p_t[:, half * RPC + ic * 128:
                                               half * RPC + (ic + 1) * 128]
                                    nc.tensor.matmul(
                                        acc[:, off:off + D + 1], lhsT, rhs,
                                        start=(first and half == 0
                                               and ic % 3 == 0),
                                        stop=(last and half == 1
                                              and (ic % 3 == 2
                                                   or ic == ICH - 1)),
                                    )

                # ---- epilogue: normalize + residual + store ----
                for h in range(HPC):
                    for b3 in range(3):
                        ics = list(range(b3 * 3, min(b3 * 3 + 3, ICH)))
                        nsl = len(ics)
                        acc = acc_tiles[h * 3 + b3]
                        accv = acc[:, 0:nsl * (D + 1)].rearrange(
                            "p (s q) -> p s q", q=D + 1)
                        den = epp.tile([128, 4], f32, tag="den", name="den")
                        denv = den[:, 0:nsl].rearrange(
                            "p (s o) -> p s o", o=1)
                        nc.vector.tensor_scalar(
                            out=denv, in0=accv[:, :, D:D + 1],
                            scalar1=1e-30, scalar2=None, op0=amax)
                        rc = epp.tile([128, 4], f32, tag="rc")
                        nc.vector.reciprocal(rc[:, 0:nsl], den[:, 0:nsl])
                        for s3, ic in enumerate(ics):
                            of = epp.tile([128, D], f32, tag="of")
                            nc.vector.scalar_tensor_tensor(
                                of,
                                in0=acc[:, s3 * (D + 1):s3 * (D + 1) + D],
                                scalar=rc[:, s3:s3 + 1],
                                in1=res[:, ic * 2 * D + h * D:
                                        ic * 2 * D + (h + 1) * D],
                                op0=mult, op1=add)
                            nc.sync.dma_start(
                                out=outb[ic * 128:(ic + 1) * 128,
                                         h * D:(h + 1) * D],
                                in_=of)
            for a_free in reversed(acc_frees):
                a_free()

    nc.compile()
    return nc


def _get_program():
    key = ("prog3",
           os.environ.get("KV2_E", "0"),
           os.environ.get("KV2_R", "13"),
           os.environ.get("KV2_GPS", "0"),
           os.environ.get("KV2_SUPP", "2"))
    if key not in _cache:
        _cache[key] = _build_program()
    return _cache[key]


def kernel(inputs, adjacency, weight, weight_u, weight_v, bias, proj_w, proj_b):
    from concourse.bass_utils import run_bass_kernel_spmd

    bf = ml_dtypes.bfloat16
    inputs = np.ascontiguousarray(np.asarray(inputs, np.float32))
    adjacency = np.asarray(adjacency, np.float32)
    weight = np.asarray(weight, np.float32)
    weight_u = np.asarray(weight_u, np.float32)
    weight_v = np.asarray(weight_v, np.float32)
    bias = np.asarray(bias, np.float32).reshape(1, H * D)
    proj_w = np.asarray(proj_w, np.float32)
    proj_b = np.asarray(proj_b, np.float32).reshape(H * D)

    nc = _get_program()

    in_maps = []
    for c in range(NCORES):
        h2 = c // 4           # head pair: heads 2*h2, 2*h2+1
        r0 = (c % 4) * RPC
        hs = slice(2 * h2 * D, (2 * h2 + 2) * D)
        rolled = np.roll(inputs, -r0, axis=0)
        inpT_c = np.ascontiguousarray(rolled.T).astype(bf)
        adjT_c = np.ascontiguousarray(
            np.roll(adjacency[r0:r0 + RPC, :], -r0, axis=1).T
        ).astype(bf)  # exact: adjacency is 0.0/1.0
        wh2_c = np.ascontiguousarray(weight[:, hs]).astype(bf)
        whT2_c = np.ascontiguousarray(
            np.concatenate([weight[:, 2 * h2 * D:(2 * h2 + 1) * D].T,
                            weight[:, (2 * h2 + 1) * D:(2 * h2 + 2) * D].T],
                           axis=1)).astype(bf)
        uv4_c = np.ascontiguousarray(
            np.concatenate([weight_u[2 * h2], weight_v[2 * h2],
                            weight_u[2 * h2 + 1], weight_v[2 * h2 + 1]],
                           axis=1)).astype(bf)
        in_maps.append({
            "adjT": adjT_c,
            "inpT": inpT_c,
            "wh2": wh2_c,
            "whT2": whT2_c,
            "uv4": uv4_c,
            "pwh": np.ascontiguousarray(proj_w[:, hs]).astype(bf),
            "br": np.ascontiguousarray(
                np.stack([bias[0, hs], proj_b[hs]], axis=0)),
        })

    trace = os.environ.get("KERNEL_TRACE", "0") == "1"
    results = run_bass_kernel_spmd(
        nc, in_maps, core_ids=list(range(NCORES)), trace=trace)
    _cache["last_results"] = results

    out = np.empty((N, H * D), np.float32)
    for c in range(NCORES):
        h2 = c // 4
        r0 = (c % 4) * RPC
        out[r0:r0 + RPC, 2 * h2 * D:(2 * h2 + 2) * D] = results.results[c]["outb"]
    return out
